# revision 12
# baseline (speedup 1.0000x reference)
"""GQA attention layer (B=2, S=2048, HID=4096, 32 Q heads / 8 KV heads, RoPE,
causal) on 8 TRN2 NeuronCores.

Strategy (tensor-parallel over heads):
  - core c owns Q heads 4c..4c+3 and KV head c (one full GQA group).
  - host pre-transposes x and the weight shards so every on-chip matmul
    contracts over the partition axis with no on-chip transposes of x.
  - projections (bf16) -> feature-major Q^T/K^T/V^T; RoPE fused into the
    PSUM eviction (DVE); attention entirely in S^T/k-major layout (fp32r
    matmuls, exp on ACT, rowsum via DVE accumulation + ones-matmul);
  - AllGather of the local ctx^T (bf16) across cores, then each core
    computes a 512-column slice of o_proj output (bf16 matmuls) and the
    host concatenates the slices.
"""

import os

import numpy as np
import ml_dtypes

B, S, HID = 2, 2048, 4096
NH, NKV, D = 32, 8, 128
T = B * S            # 4096 flattened tokens
NQ = 512             # per-core q features (4 heads x 128)
P = 128
TOKB = 512           # token block (matmul moving free dim)
NB = T // TOKB       # 8 token blocks
KC = HID // P        # 32 contraction chunks for projections
QBS = S // TOKB      # 4 q blocks per batch
KTS = S // P         # 16 k chunks per batch
SCALE = 1.0 / float(np.sqrt(np.float32(D)))
N_CORES = 8

_BUILT = None
LAST_RESULTS = None


def _build():
    from contextlib import ExitStack

    import concourse.tile as tile
    from concourse import bacc, mybir
    from concourse.masks import make_identity

    f32 = mybir.dt.float32
    f32r = mybir.dt.float32r
    bf16 = mybir.dt.bfloat16
    Exp = mybir.ActivationFunctionType.Exp

    nc = bacc.Bacc(
        "TRN2",
        target_bir_lowering=False,
        debug=False,
        num_devices=N_CORES,
    )

    xT = nc.declare_dram_parameter("xT", [HID, T], bf16, isOutput=False)
    wqT = nc.declare_dram_parameter("wqT", [HID, NQ], bf16, isOutput=False)
    wkT = nc.declare_dram_parameter("wkT", [HID, D], bf16, isOutput=False)
    wvT = nc.declare_dram_parameter("wvT", [HID, D], bf16, isOutput=False)
    woT = nc.declare_dram_parameter("woT", [HID, NQ], bf16, isOutput=False)
    cosT = nc.declare_dram_parameter("cosT", [64, T], f32, isOutput=False)
    sinT = nc.declare_dram_parameter("sinT", [64, T], f32, isOutput=False)
    maskT = nc.declare_dram_parameter("maskT", [P, 4 * TOKB], f32, isOutput=False)
    outT = nc.declare_dram_parameter("outT", [NQ, T], f32, isOutput=True)

    with tile.TileContext(nc) as tc, ExitStack() as gctx:
        ec = gctx.enter_context
        # ---- global pools (whole-kernel lifetime) ----
        dram = ec(tc.tile_pool(name="dram", bufs=1, space="DRAM"))
        const_pool = ec(tc.tile_pool(name="const_sb", bufs=1))
        ps_pool = ec(tc.tile_pool(name="ps", bufs=3, space="PSUM"))
        ctxp_pool = ec(tc.tile_pool(name="ctxp", bufs=2, space="PSUM"))
        vtp_pool = ec(tc.tile_pool(name="vtp", bufs=1, space="PSUM"))
        rs_pool = ec(tc.tile_pool(name="rsp", bufs=1, space="PSUM"))

        qt_dram = dram.tile([NQ, T], f32r)
        kt_dram = dram.tile([D, T], f32r)
        vt_dram = dram.tile([D, T], f32r)
        ag_in = dram.tile([NQ, T], bf16)
        ag_out = dram.tile([NH * D, T], bf16, addr_space="Shared")

        ones_col = const_pool.tile([P, 1], f32, name="ones_col")
        nc.vector.memset(ones_col[:, :], 1.0)
        ones_row = const_pool.tile([1, P], f32, name="ones_row")
        nc.vector.memset(ones_row[:, :], 1.0)
        ident_f32 = const_pool.tile([P, P], f32, name="ident_f32")
        make_identity(nc, ident_f32[:, :])
        ident = const_pool.tile([P, P], f32r, name="ident")
        nc.vector.tensor_copy(ident[:, :], ident_f32[:, :])

        # ================= Phase A: projections + RoPE =================
        with ExitStack() as actx:
            aec = actx.enter_context
            wq_pool = aec(tc.tile_pool(name="wq_sb", bufs=KC))
            wk_pool = aec(tc.tile_pool(name="wk_sb", bufs=KC))
            wv_pool = aec(tc.tile_pool(name="wv_sb", bufs=KC))
            xt_pool = aec(tc.tile_pool(name="xt_sb", bufs=KC + 8))
            rope_pool = aec(tc.tile_pool(name="rope_sb", bufs=1))
            evict_pool = aec(tc.tile_pool(name="evict_sb", bufs=4))
            rtmp_pool = aec(tc.tile_pool(name="rtmp_sb", bufs=4))

            cos_sb = rope_pool.tile([64, T], f32, name="cos_sb")
            nc.sync.dma_start(out=cos_sb[:, :], in_=cosT[:, :])
            sin_sb = rope_pool.tile([64, T], f32, name="sin_sb")
            nc.sync.dma_start(out=sin_sb[:, :], in_=sinT[:, :])

            wq_sb, wk_sb, wv_sb = [], [], []
            for kc in range(KC):
                wqt = wq_pool.tile([P, NQ], bf16, name="wq")
                nc.sync.dma_start(out=wqt[:, :], in_=wqT[P * kc:P * (kc + 1), :])
                wq_sb.append(wqt)
                wkt = wk_pool.tile([P, D], bf16, name="wk")
                nc.sync.dma_start(out=wkt[:, :], in_=wkT[P * kc:P * (kc + 1), :])
                wk_sb.append(wkt)
                wvt = wv_pool.tile([P, D], bf16, name="wv")
                nc.sync.dma_start(out=wvt[:, :], in_=wvT[P * kc:P * (kc + 1), :])
                wv_sb.append(wvt)

            def rope_evict(psum, dest_dram, rows, cols):
                """psum [128(d), 512(tok)] -> RoPE -> SBUF -> DRAM."""
                ev = evict_pool.tile([P, TOKB], f32r, name="ev")
                c_ap = cos_sb[:, cols]
                s_ap = sin_sb[:, cols]
                p0 = psum[0:64, :]
                p1 = psum[64:128, :]
                t0 = rtmp_pool.tile([64, TOKB], f32, name="t0")
                t1 = rtmp_pool.tile([64, TOKB], f32, name="t1")
                nc.vector.tensor_mul(t0[:, :], p0, c_ap)
                nc.vector.tensor_mul(t1[:, :], p1, s_ap)
                nc.vector.tensor_sub(ev[0:64, :], t0[:, :], t1[:, :])
                t2 = rtmp_pool.tile([64, TOKB], f32, name="t2")
                t3 = rtmp_pool.tile([64, TOKB], f32, name="t3")
                nc.vector.tensor_mul(t2[:, :], p0, s_ap)
                nc.vector.tensor_mul(t3[:, :], p1, c_ap)
                nc.vector.tensor_add(ev[64:128, :], t2[:, :], t3[:, :])
                nc.sync.dma_start(out=dest_dram[rows, cols], in_=ev[:, :])

            for nb in range(NB):
                cols = slice(TOKB * nb, TOKB * (nb + 1))
                xts = []
                for kc in range(KC):
                    xt_t = xt_pool.tile([P, TOKB], bf16, name="xt")
                    nc.sync.dma_start(
                        out=xt_t[:, :], in_=xT[P * kc:P * (kc + 1), cols]
                    )
                    xts.append(xt_t)
                # Q heads
                for m in range(4):
                    psum = ps_pool.tile([P, TOKB], f32, name="ps")
                    for kc in range(KC):
                        nc.tensor.matmul(
                            psum[:, :],
                            wq_sb[kc][:, P * m:P * (m + 1)],
                            xts[kc][:, :],
                            start=(kc == 0),
                            stop=(kc == KC - 1),
                        )
                    rope_evict(psum, qt_dram, slice(P * m, P * (m + 1)), cols)
                # K
                psum = ps_pool.tile([P, TOKB], f32, name="ps")
                for kc in range(KC):
                    nc.tensor.matmul(
                        psum[:, :], wk_sb[kc][:, :], xts[kc][:, :],
                        start=(kc == 0), stop=(kc == KC - 1),
                    )
                rope_evict(psum, kt_dram, slice(0, D), cols)
                # V (no rope)
                psum = ps_pool.tile([P, TOKB], f32, name="ps")
                for kc in range(KC):
                    nc.tensor.matmul(
                        psum[:, :], wv_sb[kc][:, :], xts[kc][:, :],
                        start=(kc == 0), stop=(kc == KC - 1),
                    )
                ev = evict_pool.tile([P, TOKB], f32r, name="ev")
                nc.scalar.copy(ev[:, :], psum[:, :])
                nc.sync.dma_start(out=vt_dram[0:D, cols], in_=ev[:, :])

        # ================= Phase B: attention =================
        with ExitStack() as bctx:
            bec = bctx.enter_context
            mask_pool = bec(tc.tile_pool(name="mask_sb", bufs=1))
            kv_pool = bec(tc.tile_pool(name="kv_sb", bufs=2))
            vt_pool = bec(tc.tile_pool(name="vt_sb", bufs=2 * KTS))
            qh_pool = bec(tc.tile_pool(name="qh_sb", bufs=4))
            e_pool = bec(tc.tile_pool(name="e_sb", bufs=6))
            acc_pool = bec(tc.tile_pool(name="acc_sb", bufs=3))
            norm_pool = bec(tc.tile_pool(name="norm_sb", bufs=2))
            ctx_out_pool = bec(tc.tile_pool(name="ctx_sb", bufs=4))

            mask_sb = mask_pool.tile([P, 4 * TOKB], f32, name="mask_sb")
            nc.sync.dma_start(out=mask_sb[:, :], in_=maskT[:, :])

            for b in range(B):
                bcols = slice(S * b, S * (b + 1))
                ktb = kv_pool.tile([P, S], f32r, name="ktb")
                nc.sync.dma_start(out=ktb[:, :], in_=kt_dram[:, bcols])
                vtb = kv_pool.tile([P, S], f32r, name="vtb")
                nc.sync.dma_start(out=vtb[:, :], in_=vt_dram[:, bcols])
                # transpose V^T -> V tiles [128(k), 128(d)]
                v_sb = []
                for kt in range(KTS):
                    vps = vtp_pool.tile([P, P], f32r, name="vps")
                    nc.tensor.transpose(
                        vps[:, :], vtb[:, P * kt:P * (kt + 1)], ident[:, :]
                    )
                    vsb = vt_pool.tile([P, P], f32r, name="vsb")
                    nc.scalar.copy(vsb[:, :], vps[:, :])
                    v_sb.append(vsb)

                for h in range(4):
                    for qb in range(QBS):
                        qcols = slice(S * b + TOKB * qb, S * b + TOKB * (qb + 1))
                        qh = qh_pool.tile([P, TOKB], f32r, name="qh")
                        nc.sync.dma_start(
                            out=qh[:, :],
                            in_=qt_dram[P * h:P * (h + 1), qcols],
                        )
                        nkt = 4 * qb + 4
                        acc = acc_pool.tile([P, TOKB], f32, name="acc")
                        ctxp = ctxp_pool.tile([P, TOKB], f32, name="ctxp")
                        for kt in range(nkt):
                            sp = ps_pool.tile([P, TOKB], f32, name="ps")
                            nc.tensor.matmul(
                                sp[:, :],
                                ktb[:, P * kt:P * (kt + 1)],
                                qh[:, :],
                                start=True, stop=True,
                            )
                            e = e_pool.tile([P, TOKB], f32r, name="e")
                            nc.scalar.activation(e[:, :], sp[:, :], Exp, scale=SCALE)
                            j = kt - 4 * qb
                            if j >= 0:
                                nc.vector.tensor_mul(
                                    e[:, :], e[:, :],
                                    mask_sb[:, TOKB * j:TOKB * (j + 1)],
                                )
                            if kt == 0:
                                nc.vector.tensor_copy(acc[:, :], e[:, :])
                            else:
                                nc.vector.tensor_add(acc[:, :], acc[:, :], e[:, :])
                            nc.tensor.matmul(
                                ctxp[:, :],
                                v_sb[kt][:, :],
                                e[:, :],
                                start=(kt == 0), stop=(kt == nkt - 1),
                            )
                        # softmax denominator: ones-matmul partition reduction
                        rs = rs_pool.tile([1, TOKB], f32, name="rs")
                        nc.tensor.matmul(
                            rs[:, :],
                            ones_col[:, :],
                            acc[:, :],
                            start=True, stop=True,
                        )
                        rcp = norm_pool.tile([1, TOKB], f32, name="rcp")
                        nc.vector.reciprocal(rcp[:, :], rs[:, :])
                        # broadcast 1/rowsum across partitions: ones-column
                        # K=1 matmul -> [128, 512] PSUM, evict to SBUF
                        rbp = rs_pool.tile([P, TOKB], f32, name="rbp")
                        nc.tensor.matmul(
                            rbp[:, :],
                            ones_row[:, :],
                            rcp[:, :],
                            start=True, stop=True,
                        )
                        rbc = norm_pool.tile([P, TOKB], f32, name="rbc")
                        nc.scalar.copy(rbc[:, :], rbp[:, :])
                        ctx_sb = ctx_out_pool.tile([P, TOKB], bf16, name="ctx_sb")
                        nc.vector.tensor_mul(ctx_sb[:, :], ctxp[:, :], rbc[:, :])
                        nc.sync.dma_start(
                            out=ag_in[P * h:P * (h + 1), qcols], in_=ctx_sb[:, :]
                        )

        # ================= Phase C: AllGather ctx^T =================
        nc.gpsimd.collective_compute(
            "AllGather",
            mybir.AluOpType.bypass,
            replica_groups=[list(range(N_CORES))],
            ins=[ag_in[:, :].opt()],
            outs=[ag_out[:, :].opt()],
        )

        # ================= Phase D: o_proj slice =================
        with ExitStack() as dctx:
            dec = dctx.enter_context
            wo_pool = dec(tc.tile_pool(name="wo_sb", bufs=KC))
            cx_pool = dec(tc.tile_pool(name="cx_sb", bufs=KC + 8))
            o_out_pool = dec(tc.tile_pool(name="o_sb", bufs=4))

            wo_sb = []
            for fc in range(KC):
                wot = wo_pool.tile([P, NQ], bf16, name="wo")
                nc.sync.dma_start(out=wot[:, :], in_=woT[P * fc:P * (fc + 1), :])
                wo_sb.append(wot)
            for tb in range(NB):
                cols = slice(TOKB * tb, TOKB * (tb + 1))
                cxs = []
                for fc in range(KC):
                    cx = cx_pool.tile([P, TOKB], bf16, name="cx")
                    nc.sync.dma_start(
                        out=cx[:, :], in_=ag_out[P * fc:P * (fc + 1), cols]
                    )
                    cxs.append(cx)
                for ot in range(4):
                    psum = ps_pool.tile([P, TOKB], f32, name="ps")
                    for fc in range(KC):
                        nc.tensor.matmul(
                            psum[:, :],
                            wo_sb[fc][:, P * ot:P * (ot + 1)],
                            cxs[fc][:, :],
                            start=(fc == 0), stop=(fc == KC - 1),
                        )
                    ov = o_out_pool.tile([P, TOKB], f32, name="ov")
                    nc.scalar.copy(ov[:, :], psum[:, :])
                    nc.sync.dma_start(
                        out=outT[P * ot:P * (ot + 1), cols], in_=ov[:, :]
                    )

    nc.finalize()
    return nc


def _host_prep(x, positions, w_q, w_k, w_v, w_o):
    bf = ml_dtypes.bfloat16
    xT = np.ascontiguousarray(x.reshape(T, HID).T).astype(bf)

    half = D // 2
    inv_freq = 1.0 / (10000.0 ** (np.arange(half, dtype=np.float32) / half))
    freqs = np.outer(np.asarray(positions, np.float32), inv_freq)  # [S, 64]
    cosT1 = np.cos(freqs).T.astype(np.float32)  # [64, S]
    sinT1 = np.sin(freqs).T.astype(np.float32)
    cosT = np.ascontiguousarray(np.concatenate([cosT1] * B, axis=1))
    sinT = np.ascontiguousarray(np.concatenate([sinT1] * B, axis=1))

    dk = np.arange(P, dtype=np.int64)[:, None]
    dq = np.arange(TOKB, dtype=np.int64)[None, :]
    maskT = np.concatenate(
        [((dk + P * j) <= dq).astype(np.float32) for j in range(4)], axis=1
    )
    maskT = np.ascontiguousarray(maskT)

    in_maps = []
    for c in range(N_CORES):
        wqTc = np.ascontiguousarray(w_q[NQ * c:NQ * (c + 1), :].T).astype(bf)
        wkTc = np.ascontiguousarray(w_k[D * c:D * (c + 1), :].T).astype(bf)
        wvTc = np.ascontiguousarray(w_v[D * c:D * (c + 1), :].T).astype(bf)
        woTc = np.ascontiguousarray(w_o[NQ * c:NQ * (c + 1), :].T).astype(bf)
        in_maps.append({
            "xT": xT, "wqT": wqTc, "wkT": wkTc, "wvT": wvTc, "woT": woTc,
            "cosT": cosT, "sinT": sinT, "maskT": maskT,
        })
    return in_maps


def _ensure_ntff_hook():
    """The agent image's antenv lacks axon_hooks; synthesize it so
    run_bass_kernel_spmd(trace=True) can capture NTFF profiles."""
    import sys
    import types
    try:
        from antenv.axon_hooks import get_axon_ntff_profile_hook  # noqa: F401
        return
    except ImportError:
        pass
    import antenv
    mod = types.ModuleType("antenv.axon_hooks")
    _h = [None]
    mod.set_axon_ntff_profile_hook = lambda h: _h.__setitem__(0, h)
    mod.get_axon_ntff_profile_hook = lambda: _h[0]
    sys.modules["antenv.axon_hooks"] = mod
    antenv.axon_hooks = mod
    try:
        from trn_agent_boot.trn_boot import _ntff_profile_via_ctypes
        mod.set_axon_ntff_profile_hook(
            _ntff_profile_via_ctypes("/opt/axon/libaxon_pjrt.so")
        )
    except Exception:
        pass


def kernel(x, positions, w_q, w_k, w_v, w_o):
    global _BUILT, LAST_RESULTS
    from concourse.bass_utils import run_bass_kernel_spmd

    x = np.asarray(x)
    positions = np.asarray(positions)
    w_q = np.asarray(w_q, np.float32)
    w_k = np.asarray(w_k, np.float32)
    w_v = np.asarray(w_v, np.float32)
    w_o = np.asarray(w_o, np.float32)

    if _BUILT is None:
        _BUILT = _build()
    nc = _BUILT

    in_maps = _host_prep(x, positions, w_q, w_k, w_v, w_o)
    trace = os.environ.get("BASS_KERNEL_TRACE", "0") == "1"
    if trace:
        _ensure_ntff_hook()
    res = run_bass_kernel_spmd(
        nc, in_maps, core_ids=list(range(N_CORES)), trace=trace
    )
    LAST_RESULTS = res

    out = np.empty((T, HID), np.float32)
    for c in range(N_CORES):
        out[:, NQ * c:NQ * (c + 1)] = np.asarray(res.results[c]["outT"]).T
    return out.reshape(B, S, HID)
